# revision 9
# baseline (speedup 1.0000x reference)
"""Sparse (routed) MoE feed-forward on 8 TRN2 NeuronCores.

Expert parallelism: core e owns expert e's weights and processes only the
tokens routed to it (top-2 membership), capacity CAP per core.

On-device pipeline per core:
  1. Router on all tokens (logits via PE, softmax/top-2 via DVE/ACT).
  2. Compaction: prefix-sum matmuls give each routed token its slot; an
     is_equal outer-compare + matmul accumulates (token_id, comb, valid)
     per slot -> packed index list, no scatter needed.
  3. dma_gather pulls the routed token rows (pad slots gather row 0 with
     combine weight 0, so they contribute nothing).
  4. SwiGLU (f32r matmuls) on the compacted set; down-proj in token-major
     form; dense [CAP, D] block + the slot->token map are written out.
Host adds each core's rows into the full [N, D] output (pad slots carry
marker index NTOK and are dropped).
"""

import numpy as np

P = 128
NTOK = 2048
D = 1024
F = 2048
F2 = 2 * F
E = 8
TCH = NTOK // P   # 16
DC = D // P       # 8
FC = F // P       # 16
CAP = 640         # per-expert token capacity (mean load 512, sigma ~20)
CJ = CAP // P     # 5 gathered chunks
NMV = CAP // 2    # 320 moving-dim chunk (>=256 keeps f32r at full rate)
BIGF = 1.0e6

_CACHE = {}


def _build():
    import concourse.bacc as bacc
    import concourse.mybir as mybir
    import concourse.tile as tile
    from concourse.masks import make_identity
    from contextlib import ExitStack

    f32 = mybir.dt.float32
    f32r = mybir.dt.float32r
    i32 = mybir.dt.int32
    i16 = mybir.dt.int16
    AF = mybir.ActivationFunctionType
    ALU = mybir.AluOpType
    AX = mybir.AxisListType

    nc = bacc.Bacc("TRN2", target_bir_lowering=False, debug=False, num_devices=8)
    x_d = nc.dram_tensor("x", [NTOK, D], f32, kind="ExternalInput").ap()
    rwt_d = nc.dram_tensor("rwt", [D, E], f32, kind="ExternalInput").ap()
    gw_d = nc.dram_tensor("gw", [D, F2], f32, kind="ExternalInput").ap()
    dw_d = nc.dram_tensor("dw", [F, D], f32, kind="ExternalInput").ap()
    og_d = nc.dram_tensor("og", [CAP, D], f32, kind="ExternalOutput").ap()
    ix_d = nc.dram_tensor("idxo", [2, CAP], f32, kind="ExternalOutput").ap()

    xr_dram = x_d.rearrange("(c p) d -> c p d", p=P)
    rw_dram = rwt_d.rearrange("(c p) e -> c p e", p=P)
    gw_dram = gw_d.rearrange("(c p) f -> c p f", p=P)
    dw_dram = dw_d.rearrange("(c p) d -> c p d", p=P)

    with tile.TileContext(nc) as tc, ExitStack() as ctx:
        cpool = ctx.enter_context(tc.tile_pool(name="const", bufs=1))
        small = ctx.enter_context(tc.tile_pool(name="small", bufs=1))
        dram = ctx.enter_context(tc.tile_pool(name="dram", bufs=1, space="DRAM"))

        ident = cpool.tile([P, P], f32, tag="ident")
        make_identity(nc, ident[:])
        # U[p, y] = 1 if p < y else 0  (strict upper triangle)
        utri = cpool.tile([P, P], f32, tag="utri")
        nc.gpsimd.memset(utri[:], 0.0)
        nc.gpsimd.affine_select(
            out=utri[:], in_=utri[:], pattern=[[-1, P]],
            compare_op=ALU.is_ge, fill=1.0, base=0, channel_multiplier=1)
        ones_col = cpool.tile([P, 1], f32, tag="ones_col")
        nc.gpsimd.memset(ones_col[:], 1.0)
        ones_row = cpool.tile([1, P], f32, tag="ones_row")
        nc.gpsimd.memset(ones_row[:], 1.0)
        # iota constants
        ids_i = cpool.tile([P, TCH], i32, tag="ids_i")
        nc.gpsimd.iota(ids_i[:], pattern=[[P, TCH]], base=0, channel_multiplier=1)
        idsf = cpool.tile([P, TCH], f32, tag="idsf")
        nc.vector.tensor_copy(idsf[:], ids_i[:])
        slot_i = cpool.tile([P, CAP], i32, tag="slot_i")
        nc.gpsimd.iota(slot_i[:], pattern=[[1, CAP]], base=0, channel_multiplier=0)
        slotf = cpool.tile([P, CAP], f32, tag="slotf")
        nc.vector.tensor_copy(slotf[:], slot_i[:])

        bounce = dram.tile([3, CAP], f32, tag="bounce")

        # ------------- Phase A: router on all tokens + compaction ----------
        with tc.tile_pool(name="xr", bufs=4) as xrp, \
             tc.tile_pool(name="xt", bufs=DC) as xtp, \
             tc.tile_pool(name="ptp", bufs=4, space="PSUM") as ptp, \
             tc.tile_pool(name="plg", bufs=1, space="PSUM") as plg:
            xT = [xtp.tile([P, NTOK], f32, tag=f"xt{d}", name=f"xT{d}", bufs=1)
                  for d in range(DC)]
            for t in range(TCH):
                xi = xrp.tile([P, D], f32, tag="xr")
                nc.sync.dma_start(xi[:], xr_dram[t])
                for d in range(DC):
                    pt = ptp.tile([P, P], f32, tag="tp")
                    nc.tensor.transpose(pt[:], xi[:, d * P:(d + 1) * P], ident[:])
                    nc.any.tensor_copy(xT[d][:, t * P:(t + 1) * P], pt[:])

            rwt_sb = small.tile([P, DC, E], f32, tag="rwt")
            for d in range(DC):
                nc.sync.dma_start(rwt_sb[:, d, :], rw_dram[d])

            # logits [tokens, E]: token-stationary, experts moving. Exact f32
            # (f32r would flip near-tie top-2 picks); moving dim is only 8 so
            # the 4-cycle/row f32 rate costs nothing.
            lg = small.tile([P, TCH, E], f32, tag="lg2")
            for t in range(TCH):
                pl = ptp.tile([P, E], f32, tag="tp")
                for d in range(DC):
                    nc.tensor.matmul(
                        pl[:],
                        xT[d][:, t * P:(t + 1) * P],
                        rwt_sb[:, d, :],
                        start=(d == 0), stop=(d == DC - 1),
                    )
                nc.any.tensor_copy(lg[:, t, :], pl[:])

            # softmax + top-2; combine weight + membership mask of expert 0
            ex = small.tile([P, TCH, E], f32, tag="ex")
            nc.scalar.activation(ex[:], lg[:], AF.Exp)
            s = small.tile([P, TCH], f32, tag="s")
            nc.vector.reduce_sum(s[:], ex[:], axis=AX.X)
            rs = small.tile([P, TCH], f32, tag="rs")
            nc.vector.reciprocal(rs[:], s[:])
            m1 = small.tile([P, TCH], f32, tag="m1")
            nc.vector.reduce_max(m1[:], lg[:], axis=AX.X)
            m1b = small.tile([P, TCH, E], f32, tag="m1b")
            for e in range(E):
                nc.any.tensor_copy(m1b[:, :, e], m1[:])
            g1 = small.tile([P, TCH, E], f32, tag="g1")
            nc.vector.tensor_tensor(g1[:], lg[:], m1b[:], op=ALU.is_ge)
            lgm = small.tile([P, TCH, E], f32, tag="lgm")
            nc.vector.tensor_scalar(lgm[:], g1[:], -1e30, None, op0=ALU.mult)
            nc.vector.tensor_tensor(lgm[:], lgm[:], lg[:], op=ALU.add)
            m2 = small.tile([P, TCH], f32, tag="m2")
            nc.vector.reduce_max(m2[:], lgm[:], axis=AX.X)
            mask0 = small.tile([P, TCH], f32, tag="mask0")
            nc.vector.tensor_tensor(mask0[:], lg[:, :, 0], m2[:], op=ALU.is_ge)
            comb = small.tile([P, TCH], f32, tag="comb")
            nc.vector.tensor_tensor(comb[:], ex[:, :, 0], mask0[:], op=ALU.mult)
            nc.vector.tensor_tensor(comb[:], comb[:], rs[:], op=ALU.mult)

            # ---- slot of each routed token: pos[p,t] = prefix count
            pA = ptp.tile([P, TCH], f32, tag="tp")
            nc.tensor.matmul(pA[:], utri[:], mask0[:],
                             start=True, stop=True)
            pT = ptp.tile([TCH, 1], f32, tag="tp")
            nc.tensor.matmul(pT[:], mask0[:], ones_col[:],
                             start=True, stop=True)
            tsb = small.tile([TCH, 1], f32, tag="tsb")
            nc.any.tensor_copy(tsb[:], pT[:])
            pO = ptp.tile([TCH, 1], f32, tag="tp")
            nc.tensor.matmul(pO[:], utri[:TCH, :TCH],
                             tsb[:], start=True, stop=True)
            osb = small.tile([TCH, 1], f32, tag="osb")
            nc.any.tensor_copy(osb[:], pO[:])
            pOr = ptp.tile([1, TCH], f32, tag="tp")
            nc.tensor.transpose(pOr[:], osb[:], ident[:TCH, :TCH])
            orow = small.tile([1, TCH], f32, tag="orow")
            nc.any.tensor_copy(orow[:], pOr[:])
            pOb = ptp.tile([P, TCH], f32, tag="tp")
            nc.tensor.matmul(pOb[:], ones_row[:],
                             orow[:], start=True, stop=True)
            pAs = small.tile([P, TCH], f32, tag="pAs")
            nc.any.tensor_copy(pAs[:], pA[:])
            posm = small.tile([P, TCH], f32, tag="posm")
            nc.vector.tensor_tensor(posm[:], pAs[:], pOb[:], op=ALU.add)
            pad = small.tile([P, TCH], f32, tag="pad")
            nc.vector.tensor_scalar(pad[:], mask0[:], -BIGF, BIGF,
                                    op0=ALU.mult, op1=ALU.add)
            nc.vector.tensor_tensor(posm[:], posm[:], pad[:], op=ALU.add)

            # ---- build (token_id, comb, valid) per slot via outer-compare
            lhs3 = small.tile([P, TCH, 3], f32, tag="lhs3")
            nc.vector.tensor_copy(lhs3[:, :, 0], idsf[:])
            nc.vector.tensor_copy(lhs3[:, :, 1], comb[:])
            nc.gpsimd.memset(lhs3[:, :, 2], 1.0)
            pcc = plg.tile([3, 2, 512], f32, tag="lg")
            for t in range(TCH):
                indv = small.tile([P, CAP], f32, tag="ind", name=f"ind{t}")
                nc.vector.tensor_scalar(
                    indv[:], slotf[:], posm[:, t:t + 1], None, op0=ALU.is_equal)
                for mv in range(2):
                    nc.tensor.matmul(
                        pcc[:, mv, 0:NMV],
                        lhs3[:, t, :],
                        indv[:, mv * NMV:(mv + 1) * NMV],
                        start=(t == 0), stop=(t == TCH - 1),
                    )
            res3 = small.tile([3, 2, NMV], f32, tag="res3")
            nc.any.tensor_copy(res3[:], pcc[:, :, 0:NMV])
            r3 = res3[:].rearrange("p a b -> p (a b)")
            # slot -> (token id, valid) map for the host combine step
            nc.sync.dma_start(ix_d[0:1, :], r3[0:1, :])
            nc.sync.dma_start(ix_d[1:2, :], r3[2:3, :])
            # gather list (pad slots -> token 0) and combine weights, rewrapped
            nc.sync.dma_start(bounce[0:1, :], r3[0:1, :])
            nc.sync.dma_start(bounce[1:2, :], r3[1:2, :])

        # ------------- Phase B: gather, SwiGLU, down-proj, write out -------
        dwp = ctx.enter_context(tc.tile_pool(name="dwt", bufs=1))
        gpool = ctx.enter_context(tc.tile_pool(name="gw", bufs=2))
        sgp = ctx.enter_context(tc.tile_pool(name="sg", bufs=4))
        hp = ctx.enter_context(tc.tile_pool(name="h", bufs=FC))
        xgp = ctx.enter_context(tc.tile_pool(name="xg", bufs=1))
        xgtp = ctx.enter_context(tc.tile_pool(name="xgt", bufs=1))
        ogp = ctx.enter_context(tc.tile_pool(name="og", bufs=2))

        dwt = []
        for fi in range(FC):
            w = dwp.tile([P, D], f32r, tag=f"dw{fi}", name=f"dw{fi}", bufs=1)
            nc.sync.dma_start(w[:], dw_dram[fi].bitcast(f32r))
            dwt.append(w)

        idx16f = small.tile([16, CAP // 16], f32, tag="idx16f")
        nc.sync.dma_start(
            idx16f[:], bounce[0, :].rearrange("(s p) -> p s", p=16))
        idx16c = small.tile([16, CAP // 16], i16, tag="idx16c")
        nc.vector.tensor_copy(idx16c[:], idx16f[:])
        # the gather's 8 gpsimd cores each read their own 16-partition slice:
        # replicate the [16, CAP//16] wrap across all 128 partitions
        idx16 = small.tile([P, CAP // 16], i16, tag="idx16")
        for k in range(8):
            nc.sync.dma_start(idx16[16 * k:16 * (k + 1), :], idx16c[:])
        cg = small.tile([P, CJ], f32, tag="cg")
        nc.sync.dma_start(cg[:], bounce[1, :].rearrange("(c p) -> p c", p=P))

        with tc.tile_pool(name="ptp2", bufs=2, space="PSUM") as ptp2, \
             tc.tile_pool(name="pgu", bufs=2, space="PSUM") as pgu, \
             tc.tile_pool(name="pdn", bufs=2, space="PSUM") as pdn:
            xg = xgp.tile([P, CJ, D], f32, tag="xg")
            nc.gpsimd.dma_gather(
                out_ap=xg[:],
                in_ap=x_d,
                idxs_ap=idx16[:],
                num_idxs=CAP,
                num_idxs_reg=CAP,
                elem_size=D,
            )
            xgT = [xgtp.tile([P, CAP], f32r, tag=f"xgt{d}", name=f"xgT{d}", bufs=1)
                   for d in range(DC)]
            for c in range(CJ):
                nc.vector.tensor_scalar(
                    xg[:, c, :], xg[:, c, :], cg[:, c:c + 1], None, op0=ALU.mult)
                for d in range(DC):
                    pt = ptp2.tile([P, P], f32, tag="tp2")
                    nc.tensor.transpose(pt[:], xg[:, c, d * P:(d + 1) * P], ident[:])
                    nc.any.tensor_copy(xgT[d][:, c * P:(c + 1) * P], pt[:])

            sg = {}
            hh = {}
            # stream gw in 256-column steps; order interleaves gate/up chunks
            for fs in (0, 8, 1, 9, 2, 10, 3, 11, 4, 12, 5, 13, 6, 14, 7, 15):
                gt = gpool.tile([P, DC, 256], f32r, tag="gw")
                for d in range(DC):
                    nc.sync.dma_start(
                        gt[:, d, :], gw_dram[d, :, fs * 256:(fs + 1) * 256].bitcast(f32r))
                for f2 in range(2):
                    fcg = fs * 2 + f2
                    ps = pgu.tile([P, 2, 512], f32, tag="gu")
                    psv = ps[:, :, 0:NMV]
                    for d in range(DC):
                        for mv in range(2):
                            nc.tensor.matmul(
                                ps[:, mv, 0:NMV],
                                gt[:, d, f2 * P:(f2 + 1) * P],
                                xgT[d][:, mv * NMV:(mv + 1) * NMV],
                                start=(d == 0), stop=(d == DC - 1),
                            )
                    if fcg < FC:
                        nc.vector.tensor_scalar(
                            psv, psv, -10.0, 10.0, op0=ALU.max, op1=ALU.min)
                        t2 = sgp.tile([P, CAP], f32, tag="sg")
                        t2v = t2[:].rearrange("p (a b) -> p a b", a=2)
                        nc.scalar.activation(t2v, psv, AF.Sigmoid)
                        nc.vector.tensor_tensor(t2v, t2v, psv, op=ALU.mult)
                        sg[fcg] = t2
                    else:
                        fch = fcg - FC
                        hv = hp.tile([P, CAP], f32r, tag="h")
                        hvv = hv[:].rearrange("p (a b) -> p a b", a=2)
                        nc.vector.tensor_tensor(
                            hvv, psv, sg[fch][:].rearrange("p (a b) -> p a b", a=2),
                            op=ALU.mult)
                        hh[fch] = hv
                        del sg[fch]

            # down proj in token-major form; write packed rows out
            for c in range(CJ):
                og = ogp.tile([P, D], f32, tag="og")
                for dh in range(2):
                    po = pdn.tile([P, 512], f32, tag="dn")
                    for fi in range(FC):
                        nc.tensor.matmul(
                            po[:],
                            hh[fi][:, c * P:(c + 1) * P],
                            dwt[fi][:, dh * 512:(dh + 1) * 512],
                            start=(fi == 0), stop=(fi == FC - 1),
                        )
                    nc.any.tensor_copy(og[:, dh * 512:(dh + 1) * 512], po[:])
                nc.sync.dma_start(og_d[c * P:(c + 1) * P, :], og[:])
    return nc


def _get_nc():
    if "nc" not in _CACHE:
        nc = _build()
        nc.compile()
        _CACHE["nc"] = nc
    return _CACHE["nc"]


def _make_in_maps(x, router_w, gate_up_w, down_w):
    x = np.ascontiguousarray(x, dtype=np.float32)
    in_maps = []
    for e in range(E):
        perm = [e] + [j for j in range(E) if j != e]
        in_maps.append({
            "x": x,
            "rwt": np.ascontiguousarray(router_w[perm].T.astype(np.float32)),
            "gw": np.ascontiguousarray(gate_up_w[e], dtype=np.float32),
            "dw": np.ascontiguousarray(down_w[e], dtype=np.float32),
        })
    return in_maps


def _combine(results):
    total = np.zeros((NTOK, D), dtype=np.float32)
    for r in results:
        idx = r["idxo"][0].astype(np.int64)
        valid = r["idxo"][1] > 0.5
        total[idx[valid]] += r["og"][valid]
    return total


def kernel(x, router_w, gate_up_w, down_w):
    from concourse import bass_utils

    nc = _get_nc()
    in_maps = _make_in_maps(x, router_w, gate_up_w, down_w)
    res = bass_utils.run_bass_kernel_spmd(nc, in_maps, core_ids=list(range(E)))
    return _combine(res.results)
